# revision 12
# baseline (speedup 1.0000x reference)
"""MoE routing kernel for Trainium2 (8 NeuronCores, expert-parallel).

Strategy (v2):
  - Host: gate (sigmoid + grouped top-k) in numpy; gather tokens per expert.
  - Device (SPMD, core e): SwiGLU MLP with expert e's weights over the tokens
    routed to e (padded to the max expert load), plus a 1/8 token shard of the
    shared-expert MLP.  All matmul operands are bf16 (fp32 PSUM accumulation):
    same PE rate as fp32r but half the DMA/SBUF traffic and fast weight loads.
  - Everything is host-pre-tiled into per-chunk [128, kt, n] blocks so each
    DMA descriptor moves long contiguous runs; loads/stores are split across
    several descriptors (and engine queues) so many DMA engines run in
    parallel — this was the v1 startup/tail bottleneck.
  - A short burst of dummy matmuls on a zeroed tile warms the PE clock (HAM)
    while the first real data is still in flight.
  - Host: weighted scatter-add of expert outputs + shared output.
"""

import numpy as np
import ml_dtypes
from contextlib import ExitStack

DIM = 768
INTER = 512
E = 8
G = 4
TOPK = 2
N_CORES = 8
P = 128
KD = DIM // P    # 6 k-tiles over model dim
KI = INTER // P  # 4 k-tiles over inter dim
NSH = 2048       # shared tokens per core
BF = ml_dtypes.bfloat16

SHARED_SIZES = [256, 256, 512, 512, 512]  # == NSH; small first chunks soften the startup DMA burst
N_WARMUP = 10   # dummy matmuls to warm the PE clock during initial DMA


# ---------------------------------------------------------------- host gate
def _host_gate(x2, gate_weight, gate_bias):
    """Reproduces reference._gate in numpy f32. Returns (w [T,2], idx [T,2])."""
    T = x2.shape[0]
    logits = x2 @ gate_weight.T
    scores = 1.0 / (1.0 + np.exp(-logits, dtype=np.float32))
    s = scores + gate_bias
    sv = s.reshape(T, G, E // G)
    group_scores = sv.sum(-1)  # top-2 of 2 per group == sum
    gidx = np.argsort(-group_scores, axis=1, kind="stable")[:, :2]
    gmask = np.zeros((T, G), bool)
    gmask[np.arange(T)[:, None], gidx] = True
    masked = np.where(gmask[:, :, None], sv, -np.inf).reshape(T, E)
    idx = np.argsort(-masked, axis=1, kind="stable")[:, :TOPK]
    w = np.take_along_axis(scores, idx, axis=1)
    w = w / (w.sum(-1, keepdims=True) + 1e-6)
    return w.astype(np.float32), idx.astype(np.int32)


def _plan(total):
    """Split into <=512 chunks; keep every chunk >=256 tokens (LDW-bound tails
    are slow), sizes multiples of 32."""
    q, r = divmod(total, 512)
    if r == 0:
        return [512] * q
    if r >= 256:
        return [512] * q + [r]
    h1 = 256 + r // 2
    return [512] * (q - 1) + [h1, 512 + r - h1]


# ---------------------------------------------------------- device kernel IR
def _build_nc(routed_sizes):
    import concourse.bass as bass
    import concourse.tile as tile
    from concourse import bacc, mybir

    f32 = mybir.dt.float32
    bf16 = mybir.dt.bfloat16

    cap = sum(routed_sizes)
    # chunk schedule: (n, phase, x_off, o_off); offsets in elements
    chunks = []
    off = 0
    for n in SHARED_SIZES:
        chunks.append((n, "s", off))
        off += DIM * n
    for n in routed_sizes:
        chunks.append((n, "r", off))
        off += DIM * n
    tot_elems = off
    assert tot_elems == DIM * (NSH + cap)

    # weight layout in win (flat): name -> (kt, M, offset)
    wspecs = {}
    woff = 0
    for name, kt, M in [
        ("sw1", KD, INTER), ("sw3", KD, INTER), ("sw2", KI, DIM),
        ("w1", KD, INTER), ("w3", KD, INTER), ("w2", KI, DIM),
    ]:
        wspecs[name] = (kt, M, woff)
        woff += P * kt * M

    nc = bacc.Bacc(
        "TRN2",
        target_bir_lowering=False,
        debug=False,
        enable_asserts=False,
        num_devices=N_CORES,
    )

    xin = nc.dram_tensor("xin", [tot_elems], bf16, kind="ExternalInput").ap()
    win = nc.dram_tensor("win", [woff], bf16, kind="ExternalInput").ap()
    oout = nc.dram_tensor("oout", [tot_elems], bf16, kind="ExternalOutput").ap()

    with tile.TileContext(nc) as tc, ExitStack() as ctx:
        wpool = ctx.enter_context(tc.tile_pool(name="wpool", bufs=1))
        xpool = ctx.enter_context(tc.tile_pool(name="xpool", bufs=3))
        hpool = ctx.enter_context(tc.tile_pool(name="hpool", bufs=2))
        slpool = ctx.enter_context(tc.tile_pool(name="slpool", bufs=5))
        opool = ctx.enter_context(tc.tile_pool(name="opool", bufs=2))
        # PSUM banks: p1 x2 + p3 x3 + p2 x3 (warm shares p2) = 8 banks.
        # p3 gets 3: the h-muls (DVE, ~0.7us) trail the p3 groups and a
        # 2-deep ring made the PE wait on them.
        pp1 = ctx.enter_context(tc.tile_pool(name="pp1", bufs=2, space="PSUM"))
        pp3 = ctx.enter_context(tc.tile_pool(name="pp3", bufs=3, space="PSUM"))
        pp2 = ctx.enter_context(tc.tile_pool(name="pp2", bufs=3, space="PSUM"))

        # ---- x chunk loads: split across descriptors for DMA-engine parallelism
        xts = {}

        def issue_x(ci, eng):
            # one descriptor: the DGE already fans each descriptor out across
            # all 16 DMA engines; extra descriptors only serialize on the
            # per-queue flow-control ring.
            n, _, xoff = chunks[ci]
            t = xpool.tile([P, KD, n], bf16, tag="xt", name=f"xt{ci}")
            src = xin[xoff : xoff + P * KD * n].rearrange(
                "(p k t) -> p k t", p=P, k=KD
            )
            eng.dma_start(out=t, in_=src)
            xts[ci] = t

        def load_w(name, eng, half=None):
            kt, M, woff_ = wspecs[name]
            if half is None:
                t = wpool.tile([P, kt, M], bf16, tag=name, name=name)
                src = win[woff_ : woff_ + P * kt * M].rearrange(
                    "(p k m) -> p k m", p=P, k=kt
                )
                eng.dma_start(out=t, in_=src)
                return t
            # half-load into an existing tile (prologue latency split)
            t, s = half
            blk = 64 * kt * M
            src = win[woff_ + s * blk : woff_ + (s + 1) * blk].rearrange(
                "(p k m) -> p k m", p=64, k=kt
            )
            eng.dma_start(out=t[s * 64 : (s + 1) * 64], in_=src)
            return t

        def load_w_mm(name, eng, mblocks=None):
            # m-major layout [P, KI, KD, 128] for up-proj weights: per-m-block
            # DMAs let the first p1 group start after 1/4 of the matrix.
            kt, M, woff_ = wspecs[name]
            t = wpool.tile([P, KI, kt, P], bf16, tag=name, name=name)
            src = win[woff_ : woff_ + P * kt * M].rearrange(
                "(p m k j) -> p m k j", p=P, m=KI, k=kt
            )
            if mblocks is None:
                eng.dma_start(out=t, in_=src)
            else:
                for m, e in mblocks:
                    e.dma_start(out=t[:, m], in_=src[:, m])
            return t

        # ---- prologue.  sync (SP) and scalar (Activation) are the hardware-DGE
        # queues; gpsimd is the software path and only gets non-critical loads.
        # Critical path to the first matmul: x chunk 0 + sw1; sw1 is split
        # across both queues so both transfer concurrently.
        warm = wpool.tile([P, 512], bf16, tag="warm", name="warm")
        nc.gpsimd.memset(warm[:, :], 0.0)
        wps = pp2.tile([P, 512], f32, tag="p2", name="warmps")
        for _ in range(N_WARMUP):
            nc.tensor.matmul(wps[:, :], warm[:, 0:P], warm[:, :], start=True, stop=True)

        issue_x(0, nc.sync)
        sw1s = load_w_mm(
            "sw1", None,
            mblocks=[(0, nc.scalar), (1, nc.sync), (2, nc.scalar), (3, nc.sync)],
        )
        sw3s = wpool.tile([P, KD, INTER], bf16, tag="sw3", name="sw3")
        load_w("sw3", nc.scalar, half=(sw3s, 0))
        load_w("sw3", nc.sync, half=(sw3s, 1))
        issue_x(1, nc.sync)
        sw2s = wpool.tile([P, KI, DIM], bf16, tag="sw2", name="sw2")
        load_w("sw2", nc.scalar, half=(sw2s, 0))
        load_w("sw2", nc.sync, half=(sw2s, 1))
        wsets = {"s": (sw1s, sw3s, sw2s)}

        def phase_up(ci):
            """p1 k-groups + silu per m-tile; returns sl tiles."""
            n, phase, _ = chunks[ci]
            a1 = wsets[phase][0]
            xt = xts[ci]
            sls = []
            for m in range(KI):
                p1 = pp1.tile([P, n], f32, tag="p1", name="p1")
                for k in range(KD):
                    nc.tensor.matmul(
                        p1[:, :], a1[:, m, k, :], xt[:, k, :],
                        start=(k == 0), stop=(k == KD - 1),
                    )
                sl = slpool.tile([P, n], bf16, tag="sl", name="sl")
                nc.scalar.activation(
                    sl[:, :], p1[:, :], mybir.ActivationFunctionType.Silu
                )
                sls.append(sl)
            return sls

        def phase_gate(ci, sls):
            """p3 k-groups + h = silu(h1) * h3 per m-tile; returns h."""
            n, phase, _ = chunks[ci]
            a3 = wsets[phase][1]
            xt = xts.pop(ci)
            h = hpool.tile([P, KI, n], bf16, tag="h", name=f"h{ci}")
            for m in range(KI):
                p3 = pp3.tile([P, n], f32, tag="p3", name="p3")
                for k in range(KD):
                    nc.tensor.matmul(
                        p3[:, :], a3[:, k, m * P : (m + 1) * P], xt[:, k, :],
                        start=(k == 0), stop=(k == KD - 1),
                    )
                nc.vector.tensor_mul(h[:, m, :], sls[m], p3[:, :])
            return h

        def phase_down(ci, h):
            """p2 k-groups + psum->sbuf copy + output DMA per m2 pair."""
            n, phase, ooff = chunks[ci]
            a2 = wsets[phase][2]
            ot = opool.tile([P, KD, n], bf16, tag="ot", name=f"ot{ci}")
            for m2 in range(KD):
                p2 = pp2.tile([P, n], f32, tag="p2", name="p2")
                for k2 in range(KI):
                    nc.tensor.matmul(
                        p2[:, :], a2[:, k2, m2 * P : (m2 + 1) * P], h[:, k2, :],
                        start=(k2 == 0), stop=(k2 == KI - 1),
                    )
                nc.vector.tensor_copy(ot[:, m2, :], p2[:, :])
                if m2 % 2 == 1:
                    # drain pair (m2-1, m2): out block layout [pair][P][2][n];
                    # the last pair is split across both queues (kernel tail)
                    j = m2 // 2
                    base = ooff + j * (P * 2 * n)
                    if j < 2:
                        dst = oout[base : base + P * 2 * n].rearrange(
                            "(p a t) -> p a t", p=P, a=2
                        )
                        nc.scalar.dma_start(out=dst, in_=ot[:, 2 * j : 2 * j + 2, :])
                    else:
                        for s, eng in ((0, nc.sync), (1, nc.scalar)):
                            dst = oout[
                                base + s * 64 * 2 * n : base + (s + 1) * 64 * 2 * n
                            ].rearrange("(p a t) -> p a t", p=64, a=2)
                            eng.dma_start(
                                out=dst,
                                in_=ot[s * 64 : (s + 1) * 64, 2 * j : 2 * j + 2, :],
                            )

        # Software pipeline: chunk c+1's up-projection runs between chunk c's
        # gate and down phases, so the PE never waits on the h-tile muls.
        # Chunk 0 runs unpipelined: the startup is DMA-capacity-bound, and
        # pipelining would pull chunk 1's x forward into the prologue burst.
        nchunks = len(chunks)
        sls = phase_up(0)
        issue_x(2, nc.scalar)
        wsets.setdefault("r", [None, None, None])[0] = load_w_mm("w1", nc.gpsimd)
        phase_down(0, phase_gate(0, sls))
        sls = phase_up(1)
        for ci in range(1, nchunks):
            if ci + 2 < nchunks:
                issue_x(ci + 2, nc.sync if ci % 2 == 0 else nc.scalar)
            if ci == 1:
                wsets["r"][1] = load_w("w3", nc.gpsimd)
            elif ci == 2:
                wsets["r"][2] = load_w("w2", nc.gpsimd)
            h = phase_gate(ci, sls)
            if ci + 1 < nchunks:
                sls = phase_up(ci + 1)
            phase_down(ci, h)

    nc.compile()
    return nc, chunks


# -------------------------------------------------------------- host packing
def _blocks_from_cols(mat, sizes):
    """mat [DIM, ncols] bf16 -> flat concat of per-chunk [P, KD, n] blocks."""
    out = []
    o = 0
    for n in sizes:
        blk = np.ascontiguousarray(
            mat[:, o : o + n].reshape(KD, P, n).transpose(1, 0, 2)
        )
        out.append(blk.ravel())
        o += n
    return out


def _pack_weight(wt):
    """wt [out, in] f32 -> lhsT tile layout [P, kt, out] bf16, flattened."""
    kt = wt.shape[1] // P
    wT = wt.T.astype(BF)  # [in, out]
    return np.ascontiguousarray(
        wT.reshape(kt, P, wt.shape[0]).transpose(1, 0, 2)
    ).ravel()


def _pack_weight_mm(wt):
    """wt [INTER, DIM] f32 -> m-major lhsT layout [P, KI, KD, P] bf16, flat."""
    wT = wt.T.astype(BF)  # [DIM, INTER]
    return np.ascontiguousarray(
        wT.reshape(KD, P, KI, P).transpose(1, 2, 0, 3)
    ).ravel()


def _decode_chunk(seg, n):
    """flat bf16 chunk block [pair][P][2][n] -> [DIM, n] fp32."""
    return (
        seg.reshape(3, P, 2, n).transpose(0, 2, 1, 3).reshape(DIM, n)
        .astype(np.float32)
    )


# ------------------------------------------------------------------- driver
def kernel(x, gate_weight, gate_bias, w1, w2, w3, sw1, sw2, sw3):
    from concourse.bass_utils import run_bass_kernel_spmd

    B, S, D = x.shape
    x2 = np.ascontiguousarray(x.reshape(-1, D))
    T = x2.shape[0]
    assert T == N_CORES * NSH

    w, idx = _host_gate(x2, gate_weight, gate_bias)

    rows_per_e = [np.nonzero((idx == e).any(axis=1))[0] for e in range(E)]
    cap = max(len(r) for r in rows_per_e)
    cap = ((cap + 63) // 64) * 64
    routed_sizes = _plan(cap)

    nc, chunks = _build_nc(routed_sizes)

    x2T = np.ascontiguousarray(x2.T).astype(BF)  # [D, T]

    wflat = np.concatenate(
        [_pack_weight_mm(sw1), _pack_weight(sw3), _pack_weight(sw2),
         _pack_weight(np.zeros((INTER, DIM), np.float32)),  # placeholder w1
         _pack_weight(np.zeros((INTER, DIM), np.float32)),  # placeholder w3
         _pack_weight(np.zeros((DIM, INTER), np.float32))]  # placeholder w2
    )
    w_sz = {"w1": P * KD * INTER, "w3": P * KD * INTER, "w2": P * KI * DIM}

    # weight segment offsets in wflat (must match _build_nc order)
    offs = {}
    o = 0
    for name, kt, M in [("sw1", KD, INTER), ("sw3", KD, INTER), ("sw2", KI, DIM),
                        ("w1", KD, INTER), ("w3", KD, INTER), ("w2", KI, DIM)]:
        offs[name] = o
        o += P * kt * M

    in_maps = []
    for e in range(E):
        rows = rows_per_e[e]
        xg = np.zeros((DIM, cap), BF)
        xg[:, : len(rows)] = x2T[:, rows]
        xs = x2T[:, e * NSH : (e + 1) * NSH]
        xin = np.concatenate(
            _blocks_from_cols(xs, SHARED_SIZES) + _blocks_from_cols(xg, routed_sizes)
        )
        wf = wflat.copy()
        wf[offs["w1"] : offs["w1"] + w_sz["w1"]] = _pack_weight_mm(w1[e])
        wf[offs["w3"] : offs["w3"] + w_sz["w3"]] = _pack_weight(w3[e])
        wf[offs["w2"] : offs["w2"] + w_sz["w2"]] = _pack_weight(w2[e])
        in_maps.append({"xin": xin, "win": wf})

    r = run_bass_kernel_spmd(nc, in_maps, list(range(N_CORES)))
    globals()["LAST_RESULTS"] = r
    res = r.results

    y = np.zeros((T, D), np.float32)
    z = np.empty((T, D), np.float32)
    for e in range(E):
        rows = rows_per_e[e]
        arr = np.asarray(res[e]["oout"]).view(BF).ravel()
        # shared chunks
        o = 0
        tok = e * NSH
        for n in SHARED_SIZES:
            z[tok : tok + n] = _decode_chunk(arr[o : o + DIM * n], n).T
            tok += n
            o += DIM * n
        # routed chunks
        col = 0
        we = np.where(idx[rows, 0] == e, w[rows, 0], w[rows, 1]).astype(np.float32)
        for n in routed_sizes:
            blk = _decode_chunk(arr[o : o + DIM * n], n)  # [D, n]
            lo, hi = col, min(col + n, len(rows))
            if lo < hi:
                rr = rows[lo:hi]
                y[rr] += we[lo:hi, None] * blk[:, : hi - lo].T
            col += n
            o += DIM * n
    return (y + z).reshape(B, S, D)


# revision 13
# speedup vs baseline: 1.0199x; 1.0199x over previous
"""MoE routing kernel for Trainium2 (8 NeuronCores, expert-parallel).

Strategy (v2):
  - Host: gate (sigmoid + grouped top-k) in numpy; gather tokens per expert.
  - Device (SPMD, core e): SwiGLU MLP with expert e's weights over the tokens
    routed to e (padded to the max expert load), plus a 1/8 token shard of the
    shared-expert MLP.  All matmul operands are bf16 (fp32 PSUM accumulation):
    same PE rate as fp32r but half the DMA/SBUF traffic and fast weight loads.
  - Everything is host-pre-tiled into per-chunk [128, kt, n] blocks so each
    DMA descriptor moves long contiguous runs; loads/stores are split across
    several descriptors (and engine queues) so many DMA engines run in
    parallel — this was the v1 startup/tail bottleneck.
  - A short burst of dummy matmuls on a zeroed tile warms the PE clock (HAM)
    while the first real data is still in flight.
  - Host: weighted scatter-add of expert outputs + shared output.
"""

import numpy as np
import ml_dtypes
from contextlib import ExitStack

DIM = 768
INTER = 512
E = 8
G = 4
TOPK = 2
N_CORES = 8
P = 128
KD = DIM // P    # 6 k-tiles over model dim
KI = INTER // P  # 4 k-tiles over inter dim
NSH = 2048       # shared tokens per core
BF = ml_dtypes.bfloat16

SHARED_SIZES = [256, 512, 512, 512, 256]  # == NSH; small first (fast start) and last (fast tail) chunks
N_WARMUP = 12   # dummy matmuls to warm the PE clock during initial DMA


# ---------------------------------------------------------------- host gate
def _host_gate(x2, gate_weight, gate_bias):
    """Reproduces reference._gate in numpy f32. Returns (w [T,2], idx [T,2])."""
    T = x2.shape[0]
    logits = x2 @ gate_weight.T
    scores = 1.0 / (1.0 + np.exp(-logits, dtype=np.float32))
    s = scores + gate_bias
    sv = s.reshape(T, G, E // G)
    group_scores = sv.sum(-1)  # top-2 of 2 per group == sum
    gidx = np.argsort(-group_scores, axis=1, kind="stable")[:, :2]
    gmask = np.zeros((T, G), bool)
    gmask[np.arange(T)[:, None], gidx] = True
    masked = np.where(gmask[:, :, None], sv, -np.inf).reshape(T, E)
    idx = np.argsort(-masked, axis=1, kind="stable")[:, :TOPK]
    w = np.take_along_axis(scores, idx, axis=1)
    w = w / (w.sum(-1, keepdims=True) + 1e-6)
    return w.astype(np.float32), idx.astype(np.int32)


def _plan(total):
    """Split into <=512 chunks; keep every chunk >=256 tokens (LDW-bound tails
    are slow), sizes multiples of 32."""
    q, r = divmod(total, 512)
    if r == 0:
        return [512] * q
    if r >= 256:
        return [512] * q + [r]
    h1 = 256 + r // 2
    return [512] * (q - 1) + [h1, 512 + r - h1]


# ---------------------------------------------------------- device kernel IR
def _build_nc(routed_sizes):
    import concourse.bass as bass
    import concourse.tile as tile
    from concourse import bacc, mybir

    f32 = mybir.dt.float32
    bf16 = mybir.dt.bfloat16

    cap = sum(routed_sizes)
    # chunk schedule: (n, phase, x_off, o_off); offsets in elements
    chunks = []
    off = 0
    for n in SHARED_SIZES:
        chunks.append((n, "s", off))
        off += DIM * n
    for n in routed_sizes:
        chunks.append((n, "r", off))
        off += DIM * n
    tot_elems = off
    assert tot_elems == DIM * (NSH + cap)

    # weight layout in win (flat): name -> (kt, M, offset)
    wspecs = {}
    woff = 0
    for name, kt, M in [
        ("sw1", KD, INTER), ("sw3", KD, INTER), ("sw2", KI, DIM),
        ("w1", KD, INTER), ("w3", KD, INTER), ("w2", KI, DIM),
    ]:
        wspecs[name] = (kt, M, woff)
        woff += P * kt * M

    nc = bacc.Bacc(
        "TRN2",
        target_bir_lowering=False,
        debug=False,
        enable_asserts=False,
        num_devices=N_CORES,
    )

    xin = nc.dram_tensor("xin", [tot_elems], bf16, kind="ExternalInput").ap()
    win = nc.dram_tensor("win", [woff], bf16, kind="ExternalInput").ap()
    oout = nc.dram_tensor("oout", [tot_elems], bf16, kind="ExternalOutput").ap()

    with tile.TileContext(nc) as tc, ExitStack() as ctx:
        wpool = ctx.enter_context(tc.tile_pool(name="wpool", bufs=1))
        xpool = ctx.enter_context(tc.tile_pool(name="xpool", bufs=3))
        hpool = ctx.enter_context(tc.tile_pool(name="hpool", bufs=2))
        slpool = ctx.enter_context(tc.tile_pool(name="slpool", bufs=5))
        opool = ctx.enter_context(tc.tile_pool(name="opool", bufs=2))
        # PSUM banks: p1 x3 + p3 x2 + p2 x3 (warm shares p2) = 8 banks
        pp1 = ctx.enter_context(tc.tile_pool(name="pp1", bufs=3, space="PSUM"))
        pp3 = ctx.enter_context(tc.tile_pool(name="pp3", bufs=2, space="PSUM"))
        pp2 = ctx.enter_context(tc.tile_pool(name="pp2", bufs=3, space="PSUM"))

        # ---- x chunk loads: split across descriptors for DMA-engine parallelism
        xts = {}

        def issue_x(ci, eng):
            # one descriptor: the DGE already fans each descriptor out across
            # all 16 DMA engines; extra descriptors only serialize on the
            # per-queue flow-control ring.
            n, _, xoff = chunks[ci]
            t = xpool.tile([P, KD, n], bf16, tag="xt", name=f"xt{ci}")
            src = xin[xoff : xoff + P * KD * n].rearrange(
                "(p k t) -> p k t", p=P, k=KD
            )
            eng.dma_start(out=t, in_=src)
            xts[ci] = t

        def load_w(name, eng, half=None):
            kt, M, woff_ = wspecs[name]
            if half is None:
                t = wpool.tile([P, kt, M], bf16, tag=name, name=name)
                src = win[woff_ : woff_ + P * kt * M].rearrange(
                    "(p k m) -> p k m", p=P, k=kt
                )
                eng.dma_start(out=t, in_=src)
                return t
            # half-load into an existing tile (prologue latency split)
            t, s = half
            blk = 64 * kt * M
            src = win[woff_ + s * blk : woff_ + (s + 1) * blk].rearrange(
                "(p k m) -> p k m", p=64, k=kt
            )
            eng.dma_start(out=t[s * 64 : (s + 1) * 64], in_=src)
            return t

        def load_w_mm(name, eng, mblocks=None):
            # m-major layout [P, KI, KD, 128] for up-proj weights: per-m-block
            # DMAs let the first p1 group start after 1/4 of the matrix.
            kt, M, woff_ = wspecs[name]
            t = wpool.tile([P, KI, kt, P], bf16, tag=name, name=name)
            src = win[woff_ : woff_ + P * kt * M].rearrange(
                "(p m k j) -> p m k j", p=P, m=KI, k=kt
            )
            if mblocks is None:
                eng.dma_start(out=t, in_=src)
            else:
                for m, e in mblocks:
                    e.dma_start(out=t[:, m], in_=src[:, m])
            return t

        # ---- prologue.  sync (SP) and scalar (Activation) are the hardware-DGE
        # queues; gpsimd is the software path and only gets non-critical loads.
        # Critical path to the first matmul: x chunk 0 + sw1; sw1 is split
        # across both queues so both transfer concurrently.
        warm = wpool.tile([P, 512], bf16, tag="warm", name="warm")
        nc.gpsimd.memset(warm[:, :], 0.0)
        wps = pp2.tile([P, 512], f32, tag="p2", name="warmps")
        for _ in range(N_WARMUP):
            nc.tensor.matmul(wps[:, :], warm[:, 0:P], warm[:, :], start=True, stop=True)

        # One descriptor per transfer (the queue ring stalls the issuing
        # engine beyond ~4 in flight), ordered by first consumption:
        #   sync:   x0, sw3, x1   |   scalar: sw1, sw2, x2
        issue_x(0, nc.sync)
        sw1s = load_w_mm("sw1", nc.scalar)
        sw3s = load_w("sw3", nc.sync)
        sw2s = load_w("sw2", nc.scalar)
        issue_x(1, nc.sync)
        wsets = {"s": (sw1s, sw3s, sw2s)}

        def phase_up(ci):
            """p1 k-groups + silu per m-tile; returns sl tiles."""
            n, phase, _ = chunks[ci]
            a1 = wsets[phase][0]
            xt = xts[ci]
            sls = []
            for m in range(KI):
                p1 = pp1.tile([P, n], f32, tag="p1", name="p1")
                for k in range(KD):
                    nc.tensor.matmul(
                        p1[:, :], a1[:, m, k, :], xt[:, k, :],
                        start=(k == 0), stop=(k == KD - 1),
                    )
                sl = slpool.tile([P, n], bf16, tag="sl", name="sl")
                nc.scalar.activation(
                    sl[:, :], p1[:, :], mybir.ActivationFunctionType.Silu
                )
                sls.append(sl)
            return sls

        def phase_gate(ci, sls):
            """p3 k-groups + h = silu(h1) * h3 per m-tile; returns h."""
            n, phase, _ = chunks[ci]
            a3 = wsets[phase][1]
            xt = xts.pop(ci)
            h = hpool.tile([P, KI, n], bf16, tag="h", name=f"h{ci}")
            for m in range(KI):
                p3 = pp3.tile([P, n], f32, tag="p3", name="p3")
                for k in range(KD):
                    nc.tensor.matmul(
                        p3[:, :], a3[:, k, m * P : (m + 1) * P], xt[:, k, :],
                        start=(k == 0), stop=(k == KD - 1),
                    )
                nc.vector.tensor_mul(h[:, m, :], sls[m], p3[:, :])
            return h

        def phase_down(ci, h):
            """p2 k-groups + psum->sbuf copy + output DMA per m2 pair."""
            n, phase, ooff = chunks[ci]
            a2 = wsets[phase][2]
            ot = opool.tile([P, KD, n], bf16, tag="ot", name=f"ot{ci}")
            for m2 in range(KD):
                p2 = pp2.tile([P, n], f32, tag="p2", name="p2")
                for k2 in range(KI):
                    nc.tensor.matmul(
                        p2[:, :], a2[:, k2, m2 * P : (m2 + 1) * P], h[:, k2, :],
                        start=(k2 == 0), stop=(k2 == KI - 1),
                    )
                nc.vector.tensor_copy(ot[:, m2, :], p2[:, :])
                if m2 % 2 == 1:
                    # drain pair (m2-1, m2): out block layout [pair][P][2][n]
                    j = m2 // 2
                    dst = oout[
                        ooff + j * (P * 2 * n) : ooff + (j + 1) * (P * 2 * n)
                    ].rearrange("(p a t) -> p a t", p=P, a=2)
                    eng = nc.scalar if j < 2 else nc.sync
                    eng.dma_start(out=dst, in_=ot[:, 2 * j : 2 * j + 2, :])

        # Software pipeline: chunk c+1's up-projection runs between chunk c's
        # gate and down phases, so the PE never waits on the h-tile muls.
        nchunks = len(chunks)
        sls = phase_up(0)
        for ci in range(nchunks):
            if ci + 2 < nchunks:
                issue_x(ci + 2, nc.scalar if ci % 2 == 0 else nc.sync)
            if ci == 0:
                wsets.setdefault("r", [None, None, None])[0] = load_w_mm("w1", nc.gpsimd)
            elif ci == 1:
                wsets["r"][1] = load_w("w3", nc.gpsimd)
            elif ci == 2:
                wsets["r"][2] = load_w("w2", nc.gpsimd)
            h = phase_gate(ci, sls)
            if ci + 1 < nchunks:
                sls = phase_up(ci + 1)
            phase_down(ci, h)

    nc.compile()
    return nc, chunks


# -------------------------------------------------------------- host packing
def _blocks_from_cols(mat, sizes):
    """mat [DIM, ncols] bf16 -> flat concat of per-chunk [P, KD, n] blocks."""
    out = []
    o = 0
    for n in sizes:
        blk = np.ascontiguousarray(
            mat[:, o : o + n].reshape(KD, P, n).transpose(1, 0, 2)
        )
        out.append(blk.ravel())
        o += n
    return out


def _pack_weight(wt):
    """wt [out, in] f32 -> lhsT tile layout [P, kt, out] bf16, flattened."""
    kt = wt.shape[1] // P
    wT = wt.T.astype(BF)  # [in, out]
    return np.ascontiguousarray(
        wT.reshape(kt, P, wt.shape[0]).transpose(1, 0, 2)
    ).ravel()


def _pack_weight_mm(wt):
    """wt [INTER, DIM] f32 -> m-major lhsT layout [P, KI, KD, P] bf16, flat."""
    wT = wt.T.astype(BF)  # [DIM, INTER]
    return np.ascontiguousarray(
        wT.reshape(KD, P, KI, P).transpose(1, 2, 0, 3)
    ).ravel()


def _decode_chunk(seg, n):
    """flat bf16 chunk block [pair][P][2][n] -> [DIM, n] fp32."""
    return (
        seg.reshape(3, P, 2, n).transpose(0, 2, 1, 3).reshape(DIM, n)
        .astype(np.float32)
    )


# ------------------------------------------------------------------- driver
def kernel(x, gate_weight, gate_bias, w1, w2, w3, sw1, sw2, sw3):
    from concourse.bass_utils import run_bass_kernel_spmd

    B, S, D = x.shape
    x2 = np.ascontiguousarray(x.reshape(-1, D))
    T = x2.shape[0]
    assert T == N_CORES * NSH

    w, idx = _host_gate(x2, gate_weight, gate_bias)

    rows_per_e = [np.nonzero((idx == e).any(axis=1))[0] for e in range(E)]
    cap = max(len(r) for r in rows_per_e)
    cap = ((cap + 63) // 64) * 64
    routed_sizes = _plan(cap)

    nc, chunks = _build_nc(routed_sizes)

    x2T = np.ascontiguousarray(x2.T).astype(BF)  # [D, T]

    wflat = np.concatenate(
        [_pack_weight_mm(sw1), _pack_weight(sw3), _pack_weight(sw2),
         _pack_weight(np.zeros((INTER, DIM), np.float32)),  # placeholder w1
         _pack_weight(np.zeros((INTER, DIM), np.float32)),  # placeholder w3
         _pack_weight(np.zeros((DIM, INTER), np.float32))]  # placeholder w2
    )
    w_sz = {"w1": P * KD * INTER, "w3": P * KD * INTER, "w2": P * KI * DIM}

    # weight segment offsets in wflat (must match _build_nc order)
    offs = {}
    o = 0
    for name, kt, M in [("sw1", KD, INTER), ("sw3", KD, INTER), ("sw2", KI, DIM),
                        ("w1", KD, INTER), ("w3", KD, INTER), ("w2", KI, DIM)]:
        offs[name] = o
        o += P * kt * M

    in_maps = []
    for e in range(E):
        rows = rows_per_e[e]
        xg = np.zeros((DIM, cap), BF)
        xg[:, : len(rows)] = x2T[:, rows]
        xs = x2T[:, e * NSH : (e + 1) * NSH]
        xin = np.concatenate(
            _blocks_from_cols(xs, SHARED_SIZES) + _blocks_from_cols(xg, routed_sizes)
        )
        wf = wflat.copy()
        wf[offs["w1"] : offs["w1"] + w_sz["w1"]] = _pack_weight_mm(w1[e])
        wf[offs["w3"] : offs["w3"] + w_sz["w3"]] = _pack_weight(w3[e])
        wf[offs["w2"] : offs["w2"] + w_sz["w2"]] = _pack_weight(w2[e])
        in_maps.append({"xin": xin, "win": wf})

    r = run_bass_kernel_spmd(nc, in_maps, list(range(N_CORES)))
    globals()["LAST_RESULTS"] = r
    res = r.results

    y = np.zeros((T, D), np.float32)
    z = np.empty((T, D), np.float32)
    for e in range(E):
        rows = rows_per_e[e]
        arr = np.asarray(res[e]["oout"]).view(BF).ravel()
        # shared chunks
        o = 0
        tok = e * NSH
        for n in SHARED_SIZES:
            z[tok : tok + n] = _decode_chunk(arr[o : o + DIM * n], n).T
            tok += n
            o += DIM * n
        # routed chunks
        col = 0
        we = np.where(idx[rows, 0] == e, w[rows, 0], w[rows, 1]).astype(np.float32)
        for n in routed_sizes:
            blk = _decode_chunk(arr[o : o + DIM * n], n)  # [D, n]
            lo, hi = col, min(col + n, len(rows))
            if lo < hi:
                rr = rows[lo:hi]
                y[rr] += we[lo:hi, None] * blk[:, : hi - lo].T
            col += n
            o += DIM * n
    return (y + z).reshape(B, S, D)


# revision 14
# speedup vs baseline: 1.0275x; 1.0075x over previous
"""MoE routing kernel for Trainium2 (8 NeuronCores, expert-parallel).

Strategy (v2):
  - Host: gate (sigmoid + grouped top-k) in numpy; gather tokens per expert.
  - Device (SPMD, core e): SwiGLU MLP with expert e's weights over the tokens
    routed to e (padded to the max expert load), plus a 1/8 token shard of the
    shared-expert MLP.  All matmul operands are bf16 (fp32 PSUM accumulation):
    same PE rate as fp32r but half the DMA/SBUF traffic and fast weight loads.
  - Everything is host-pre-tiled into per-chunk [128, kt, n] blocks so each
    DMA descriptor moves long contiguous runs; loads/stores are split across
    several descriptors (and engine queues) so many DMA engines run in
    parallel — this was the v1 startup/tail bottleneck.
  - A short burst of dummy matmuls on a zeroed tile warms the PE clock (HAM)
    while the first real data is still in flight.
  - Host: weighted scatter-add of expert outputs + shared output.
"""

import numpy as np
import ml_dtypes
from contextlib import ExitStack

DIM = 768
INTER = 512
E = 8
G = 4
TOPK = 2
N_CORES = 8
P = 128
KD = DIM // P    # 6 k-tiles over model dim
KI = INTER // P  # 4 k-tiles over inter dim
NSH = 2048       # shared tokens per core
BF = ml_dtypes.bfloat16

SHARED_SIZES = [512, 512, 512, 512]  # == NSH
N_WARMUP = 10   # dummy matmuls to warm the PE clock during initial DMA


# ---------------------------------------------------------------- host gate
def _host_gate(x2, gate_weight, gate_bias):
    """Reproduces reference._gate in numpy f32. Returns (w [T,2], idx [T,2])."""
    T = x2.shape[0]
    logits = x2 @ gate_weight.T
    scores = 1.0 / (1.0 + np.exp(-logits, dtype=np.float32))
    s = scores + gate_bias
    sv = s.reshape(T, G, E // G)
    group_scores = sv.sum(-1)  # top-2 of 2 per group == sum
    gidx = np.argsort(-group_scores, axis=1, kind="stable")[:, :2]
    gmask = np.zeros((T, G), bool)
    gmask[np.arange(T)[:, None], gidx] = True
    masked = np.where(gmask[:, :, None], sv, -np.inf).reshape(T, E)
    idx = np.argsort(-masked, axis=1, kind="stable")[:, :TOPK]
    w = np.take_along_axis(scores, idx, axis=1)
    w = w / (w.sum(-1, keepdims=True) + 1e-6)
    return w.astype(np.float32), idx.astype(np.int32)


def _plan(total):
    """Split into <=512 chunks; keep every chunk >=256 tokens (LDW-bound tails
    are slow), sizes multiples of 32."""
    q, r = divmod(total, 512)
    if r == 0:
        return [512] * q
    if r >= 256:
        return [512] * q + [r]
    h1 = 256 + r // 2
    return [512] * (q - 1) + [h1, 512 + r - h1]


# ---------------------------------------------------------- device kernel IR
def _build_nc(routed_sizes):
    import concourse.bass as bass
    import concourse.tile as tile
    from concourse import bacc, mybir

    f32 = mybir.dt.float32
    bf16 = mybir.dt.bfloat16

    cap = sum(routed_sizes)
    # chunk schedule: (n, phase, x_off, o_off); offsets in elements
    chunks = []
    off = 0
    for n in SHARED_SIZES:
        chunks.append((n, "s", off))
        off += DIM * n
    for n in routed_sizes:
        chunks.append((n, "r", off))
        off += DIM * n
    tot_elems = off
    assert tot_elems == DIM * (NSH + cap)

    # weight layout in win (flat): name -> (kt, M, offset)
    wspecs = {}
    woff = 0
    for name, kt, M in [
        ("sw1", KD, INTER), ("sw3", KD, INTER), ("sw2", KI, DIM),
        ("w1", KD, INTER), ("w3", KD, INTER), ("w2", KI, DIM),
    ]:
        wspecs[name] = (kt, M, woff)
        woff += P * kt * M

    nc = bacc.Bacc(
        "TRN2",
        target_bir_lowering=False,
        debug=False,
        enable_asserts=False,
        num_devices=N_CORES,
    )

    xin = nc.dram_tensor("xin", [tot_elems], bf16, kind="ExternalInput").ap()
    win = nc.dram_tensor("win", [woff], bf16, kind="ExternalInput").ap()
    oout = nc.dram_tensor("oout", [tot_elems], bf16, kind="ExternalOutput").ap()

    with tile.TileContext(nc) as tc, ExitStack() as ctx:
        wpool = ctx.enter_context(tc.tile_pool(name="wpool", bufs=1))
        xpool = ctx.enter_context(tc.tile_pool(name="xpool", bufs=3))
        hpool = ctx.enter_context(tc.tile_pool(name="hpool", bufs=2))
        slpool = ctx.enter_context(tc.tile_pool(name="slpool", bufs=5))
        opool = ctx.enter_context(tc.tile_pool(name="opool", bufs=2))
        # PSUM banks: p1 x3 + p3 x2 + p2 x3 (warm shares p2) = 8 banks
        pp1 = ctx.enter_context(tc.tile_pool(name="pp1", bufs=3, space="PSUM"))
        pp3 = ctx.enter_context(tc.tile_pool(name="pp3", bufs=2, space="PSUM"))
        pp2 = ctx.enter_context(tc.tile_pool(name="pp2", bufs=3, space="PSUM"))

        # ---- x chunk loads: split across descriptors for DMA-engine parallelism
        xts = {}

        def issue_x(ci, eng):
            # one descriptor: the DGE already fans each descriptor out across
            # all 16 DMA engines; extra descriptors only serialize on the
            # per-queue flow-control ring.
            n, _, xoff = chunks[ci]
            t = xpool.tile([P, KD, n], bf16, tag="xt", name=f"xt{ci}")
            src = xin[xoff : xoff + P * KD * n].rearrange(
                "(p k t) -> p k t", p=P, k=KD
            )
            eng.dma_start(out=t, in_=src)
            xts[ci] = t

        def load_w(name, eng, half=None):
            kt, M, woff_ = wspecs[name]
            if half is None:
                t = wpool.tile([P, kt, M], bf16, tag=name, name=name)
                src = win[woff_ : woff_ + P * kt * M].rearrange(
                    "(p k m) -> p k m", p=P, k=kt
                )
                eng.dma_start(out=t, in_=src)
                return t
            # half-load into an existing tile (prologue latency split)
            t, s = half
            blk = 64 * kt * M
            src = win[woff_ + s * blk : woff_ + (s + 1) * blk].rearrange(
                "(p k m) -> p k m", p=64, k=kt
            )
            eng.dma_start(out=t[s * 64 : (s + 1) * 64], in_=src)
            return t

        # ---- prologue.  sync (SP) and scalar (Activation) are the hardware-DGE
        # queues; gpsimd is the software path and only gets non-critical loads.
        # Critical path to the first matmul: x chunk 0 + sw1; sw1 is split
        # across both queues so both transfer concurrently.
        warm = wpool.tile([P, 512], bf16, tag="warm", name="warm")
        nc.gpsimd.memset(warm[:, :], 0.0)
        wps = pp2.tile([P, 512], f32, tag="p2", name="warmps")
        for _ in range(N_WARMUP):
            nc.tensor.matmul(wps[:, :], warm[:, 0:P], warm[:, :], start=True, stop=True)

        # One descriptor per transfer (the queue ring stalls the issuing
        # engine beyond ~4 in flight); sw1 halves go on both queues so the
        # first matmul's dependencies transfer concurrently.
        sw1s = wpool.tile([P, KD, INTER], bf16, tag="sw1", name="sw1")
        issue_x(0, nc.sync)
        load_w("sw1", nc.scalar, half=(sw1s, 0))
        load_w("sw1", nc.sync, half=(sw1s, 1))
        sw3s = load_w("sw3", nc.scalar)
        sw2s = load_w("sw2", nc.sync)
        issue_x(1, nc.scalar)
        wsets = {"s": (sw1s, sw3s, sw2s)}

        def phase_up(ci):
            """p1 k-groups + silu per m-tile; returns sl tiles."""
            n, phase, _ = chunks[ci]
            a1 = wsets[phase][0]
            xt = xts[ci]
            sls = []
            for m in range(KI):
                p1 = pp1.tile([P, n], f32, tag="p1", name="p1")
                for k in range(KD):
                    nc.tensor.matmul(
                        p1[:, :], a1[:, k, m * P : (m + 1) * P], xt[:, k, :],
                        start=(k == 0), stop=(k == KD - 1),
                    )
                sl = slpool.tile([P, n], bf16, tag="sl", name="sl")
                nc.scalar.activation(
                    sl[:, :], p1[:, :], mybir.ActivationFunctionType.Silu
                )
                sls.append(sl)
            return sls

        def phase_gate(ci, sls):
            """p3 k-groups + h = silu(h1) * h3 per m-tile; returns h."""
            n, phase, _ = chunks[ci]
            a3 = wsets[phase][1]
            xt = xts.pop(ci)
            h = hpool.tile([P, KI, n], bf16, tag="h", name=f"h{ci}")
            for m in range(KI):
                p3 = pp3.tile([P, n], f32, tag="p3", name="p3")
                for k in range(KD):
                    nc.tensor.matmul(
                        p3[:, :], a3[:, k, m * P : (m + 1) * P], xt[:, k, :],
                        start=(k == 0), stop=(k == KD - 1),
                    )
                nc.vector.tensor_mul(h[:, m, :], sls[m], p3[:, :])
            return h

        def phase_down(ci, h):
            """p2 k-groups + psum->sbuf copy + output DMA per m2 pair."""
            n, phase, ooff = chunks[ci]
            a2 = wsets[phase][2]
            ot = opool.tile([P, KD, n], bf16, tag="ot", name=f"ot{ci}")
            for m2 in range(KD):
                p2 = pp2.tile([P, n], f32, tag="p2", name="p2")
                for k2 in range(KI):
                    nc.tensor.matmul(
                        p2[:, :], a2[:, k2, m2 * P : (m2 + 1) * P], h[:, k2, :],
                        start=(k2 == 0), stop=(k2 == KI - 1),
                    )
                # copy on ACT: keeps the vector queue free for the h-muls,
                # which gate the next chunk's p3 groups
                nc.scalar.activation(
                    ot[:, m2, :], p2[:, :], mybir.ActivationFunctionType.Copy
                )
                if m2 % 2 == 1:
                    # drain pair (m2-1, m2): out block layout [pair][P][2][n]
                    j = m2 // 2
                    dst = oout[
                        ooff + j * (P * 2 * n) : ooff + (j + 1) * (P * 2 * n)
                    ].rearrange("(p a t) -> p a t", p=P, a=2)
                    nc.sync.dma_start(out=dst, in_=ot[:, 2 * j : 2 * j + 2, :])

        # Software pipeline: chunk c+1's up-projection runs between chunk c's
        # gate and down phases, so the PE never waits on the h-tile muls.
        nchunks = len(chunks)
        sls = phase_up(0)
        for ci in range(nchunks):
            if ci + 2 < nchunks:
                issue_x(ci + 2, nc.sync if ci % 2 == 0 else nc.scalar)
            if ci == 0:
                wsets.setdefault("r", [None, None, None])[0] = load_w("w1", nc.gpsimd)
            elif ci == 1:
                wsets["r"][1] = load_w("w3", nc.gpsimd)
            elif ci == 2:
                wsets["r"][2] = load_w("w2", nc.gpsimd)
            h = phase_gate(ci, sls)
            if ci + 1 < nchunks:
                sls = phase_up(ci + 1)
            phase_down(ci, h)

    nc.compile()
    return nc, chunks


# -------------------------------------------------------------- host packing
def _blocks_from_cols(mat, sizes):
    """mat [DIM, ncols] bf16 -> flat concat of per-chunk [P, KD, n] blocks."""
    out = []
    o = 0
    for n in sizes:
        blk = np.ascontiguousarray(
            mat[:, o : o + n].reshape(KD, P, n).transpose(1, 0, 2)
        )
        out.append(blk.ravel())
        o += n
    return out


def _pack_weight(wt):
    """wt [out, in] f32 -> lhsT tile layout [P, kt, out] bf16, flattened."""
    kt = wt.shape[1] // P
    wT = wt.T.astype(BF)  # [in, out]
    return np.ascontiguousarray(
        wT.reshape(kt, P, wt.shape[0]).transpose(1, 0, 2)
    ).ravel()


def _pack_weight_mm(wt):
    """wt [INTER, DIM] f32 -> m-major lhsT layout [P, KI, KD, P] bf16, flat."""
    wT = wt.T.astype(BF)  # [DIM, INTER]
    return np.ascontiguousarray(
        wT.reshape(KD, P, KI, P).transpose(1, 2, 0, 3)
    ).ravel()


def _decode_chunk(seg, n):
    """flat bf16 chunk block [pair][P][2][n] -> [DIM, n] fp32."""
    return (
        seg.reshape(3, P, 2, n).transpose(0, 2, 1, 3).reshape(DIM, n)
        .astype(np.float32)
    )


# ------------------------------------------------------------------- driver
def kernel(x, gate_weight, gate_bias, w1, w2, w3, sw1, sw2, sw3):
    from concourse.bass_utils import run_bass_kernel_spmd

    B, S, D = x.shape
    x2 = np.ascontiguousarray(x.reshape(-1, D))
    T = x2.shape[0]
    assert T == N_CORES * NSH

    w, idx = _host_gate(x2, gate_weight, gate_bias)

    rows_per_e = [np.nonzero((idx == e).any(axis=1))[0] for e in range(E)]
    cap = max(len(r) for r in rows_per_e)
    cap = ((cap + 63) // 64) * 64
    routed_sizes = _plan(cap)

    nc, chunks = _build_nc(routed_sizes)

    x2T = np.ascontiguousarray(x2.T).astype(BF)  # [D, T]

    wflat = np.concatenate(
        [_pack_weight(sw1), _pack_weight(sw3), _pack_weight(sw2),
         _pack_weight(np.zeros((INTER, DIM), np.float32)),  # placeholder w1
         _pack_weight(np.zeros((INTER, DIM), np.float32)),  # placeholder w3
         _pack_weight(np.zeros((DIM, INTER), np.float32))]  # placeholder w2
    )
    w_sz = {"w1": P * KD * INTER, "w3": P * KD * INTER, "w2": P * KI * DIM}

    # weight segment offsets in wflat (must match _build_nc order)
    offs = {}
    o = 0
    for name, kt, M in [("sw1", KD, INTER), ("sw3", KD, INTER), ("sw2", KI, DIM),
                        ("w1", KD, INTER), ("w3", KD, INTER), ("w2", KI, DIM)]:
        offs[name] = o
        o += P * kt * M

    in_maps = []
    for e in range(E):
        rows = rows_per_e[e]
        xg = np.zeros((DIM, cap), BF)
        xg[:, : len(rows)] = x2T[:, rows]
        xs = x2T[:, e * NSH : (e + 1) * NSH]
        xin = np.concatenate(
            _blocks_from_cols(xs, SHARED_SIZES) + _blocks_from_cols(xg, routed_sizes)
        )
        wf = wflat.copy()
        wf[offs["w1"] : offs["w1"] + w_sz["w1"]] = _pack_weight(w1[e])
        wf[offs["w3"] : offs["w3"] + w_sz["w3"]] = _pack_weight(w3[e])
        wf[offs["w2"] : offs["w2"] + w_sz["w2"]] = _pack_weight(w2[e])
        in_maps.append({"xin": xin, "win": wf})

    r = run_bass_kernel_spmd(nc, in_maps, list(range(N_CORES)))
    globals()["LAST_RESULTS"] = r
    res = r.results

    y = np.zeros((T, D), np.float32)
    z = np.empty((T, D), np.float32)
    for e in range(E):
        rows = rows_per_e[e]
        arr = np.asarray(res[e]["oout"]).view(BF).ravel()
        # shared chunks
        o = 0
        tok = e * NSH
        for n in SHARED_SIZES:
            z[tok : tok + n] = _decode_chunk(arr[o : o + DIM * n], n).T
            tok += n
            o += DIM * n
        # routed chunks
        col = 0
        we = np.where(idx[rows, 0] == e, w[rows, 0], w[rows, 1]).astype(np.float32)
        for n in routed_sizes:
            blk = _decode_chunk(arr[o : o + DIM * n], n)  # [D, n]
            lo, hi = col, min(col + n, len(rows))
            if lo < hi:
                rr = rows[lo:hi]
                y[rr] += we[lo:hi, None] * blk[:, : hi - lo].T
            col += n
            o += DIM * n
    return (y + z).reshape(B, S, D)
